# revision 20
# baseline (speedup 1.0000x reference)
"""CrossAttentionFusion Trainium2 kernel (nn_CrossAttentionFusion__45561013076033).

Full inputs -> full output. Sharding: 8 cores, core c handles batch b=c//2,
query-half h=c%2 (2048 of 4096 queries). Each core holds the full [256,4096]
cnn feature map of its batch (keys), its query-half of the transformer
features, and replicated weights.

Key restructurings vs the naive dataflow:
  * out = Wf1 @ x_trf + Wf2 @ attended + bf'.  Fold Wf2 into the value
    projection: U = (Wf2 @ Wv) @ x_cnn, so attention directly produces
    conv-ready channels; bv's contribution is constant (softmax rows sum
    to 1) and lands in bf' = bf + Wf2 @ bv.
  * Q/K projections, scores, AND the PV matmul all run as fp8e4m3
    DoubleRow matmuls (256-deep contraction in one pass).  exp() writes
    P^T straight to fp8 (logits are bounded ~3, so exp stays far below
    the TRN e4m3 max of 240), and U is quantized to fp8 after its
    on-chip projection; softmax averaging washes the quantization out
    (measured end-to-end rel err 0.0026, same as the f16-P version).
  * Scores are computed pre-transposed, S^T[k, q] = K_kt^T Q, so the P^T
    needed by the PV matmul comes straight out of exp() -- no transpose
    of the [N, N] attention matrix.
  * PV runs with P^T tiles as the *stationary* operand pairs (DoubleRow
    over 256 keys) and U^T [keys, 257] as the moving operand: softmax
    row-sums come free as a 257th column (constant-16: cancels in A/R),
    and normalization stays a cheap per-partition reciprocal+scale.
  * The [q, e] -> [e, q] layout fix-up is a matmul with a 128x128 identity
    as the moving operand, accumulated directly into the Wf1 PSUM group
    (Wf1 runs in bf16 -- the direct conv path needs >=bf16 precision).
  * Q/K projection PSUM->SBUF moves (bias add + fp8 cast) run on the
    vector engine, keeping the scalar engine free for the exp stream
    (exp is the ACT-engine floor at ~73us/core).
"""

import numpy as np

B, C, H, W = 4, 256, 64, 64
N = H * W            # 4096 tokens
NCORES = 8
QH = N // 2          # 2048 queries per core
CT = C // 128        # 2 channel tiles
KC = N // 512        # 8 key chunks of 512
NSB = QH // 512      # 4 superblocks per core
NKT = N // 128       # 32 key tiles
NDR = NKT // 2       # 16 DoubleRow key tiles (256 keys each)
UW = 272             # padded U^T row stride (257 used; 16-aligned)

_CACHE = {}


def _build():
    import concourse.bass as bass
    import concourse.mybir as mybir
    import concourse.tile as tile
    from concourse import bacc
    from concourse.masks import make_identity

    f32 = mybir.dt.float32
    bf16 = mybir.dt.bfloat16
    f16 = mybir.dt.float16
    f8 = mybir.dt.float8e4
    AF = mybir.ActivationFunctionType
    DR = mybir.MatmulPerfMode.DoubleRow

    nc = bacc.Bacc("TRN2", target_bir_lowering=False, debug=False)

    XQ8 = nc.dram_tensor("xq8", [C, QH], f8, kind="ExternalInput")
    XC8 = nc.dram_tensor("xc8", [C, N], f8, kind="ExternalInput")
    XQB = nc.dram_tensor("xqb", [C, QH], bf16, kind="ExternalInput")
    # wq/wk/wu pre-packed on host as [128, 3, 2, 256]: one DMA with
    # 1536B partition rows (separate [256,256] tensors would move as
    # 256B rows -- ~4x the DMA time, each paying the ~2us completion
    # latency)
    WQKU = nc.dram_tensor("wqku", [128, 3, CT, C], f8, kind="ExternalInput")
    WF1 = nc.dram_tensor("wf1", [C, C], bf16, kind="ExternalInput")
    BIAS = nc.dram_tensor("bias3", [3, C], f32, kind="ExternalInput")
    OUT = nc.dram_tensor("out", [C, QH], f32, kind="ExternalOutput")

    xq8_d = XQ8.ap().rearrange("(t p) n -> p t n", p=128)
    xc8_d = XC8.ap().rearrange("(t p) n -> p t n", p=128)
    xqb_d = XQB.ap().rearrange("(t p) n -> p t n", p=128)
    wf_d = WF1.ap().rearrange("(t p) d -> p t d", p=128)
    out_d = OUT.ap().rearrange("(t p) n -> p t n", p=128)

    with tile.TileContext(nc) as tc:
        with tc.tile_pool(name="persist", bufs=1) as per, \
             tc.tile_pool(name="pt", bufs=2) as ptp, \
             tc.tile_pool(name="cb", bufs=4) as cbp, \
             tc.tile_pool(name="outp", bufs=2) as outp, \
             tc.tile_pool(name="mm", bufs=1, space="PSUM") as mmp:

            # ---- persistent tiles ----
            xq8_sb = per.tile([128, CT, QH], f8)
            xc8_sb = per.tile([128, CT, N], f8)
            xqb_sb = per.tile([128, CT, QH], bf16)
            wqku_sb = per.tile([128, 3, CT, C], f8)
            wf_sb = per.tile([128, CT, C], bf16)
            bias_sb = per.tile([128, 3, CT], f32)
            q8_sb = per.tile([128, CT, QH], f8)
            k8_sb = per.tile([128, CT, N], f8)
            ut_sb = per.tile([128, NKT, UW], f8)
            ident = per.tile([128, 128], f16)
            warm_sb = per.tile([1, 1], f32)

            # input DMAs, ordered by first consumer.  1024-column chunks
            # keep per-partition DMA rows >= 1KB (descriptor efficiency);
            # the three bias vectors ride in one packed tensor.
            # preload the exp activation table while the DMAs run (the
            # first real exp would otherwise pay the ~2.7us table switch)
            # spread the input DMAs across the three DMA-capable engine
            # queues (sync/scalar/gpsimd); each dma_start carries a ~2us
            # completion latency, so the lead-critical loads (weights,
            # xq/xc chunk0) get their own queues and everything ships in
            # few, large-row transfers.
            # lead-critical loads ride the two HWDGE queues (sync/scalar)
            # -- the gpsimd queue is SWDGE: its software descriptor
            # generation adds ~5us+ for multi-row patterns, so it only
            # carries the late-needed bulk (wf, xqb).
            nc.scalar.dma_start(wqku_sb[:], WQKU.ap())
            nc.scalar.dma_start(xq8_sb[:, :, 0:1024], xq8_d[:, :, 0:1024])
            nc.sync.dma_start(xc8_sb[:, :, 0:1024], xc8_d[:, :, 0:1024])
            nc.vector.memset(warm_sb[:], 0.0)
            nc.scalar.activation(warm_sb[:], warm_sb[:], AF.Exp, scale=0.0)
            nc.scalar.dma_start(bias_sb[:],
                                BIAS.ap().rearrange("b (t p) -> p b t", p=128))
            nc.sync.dma_start(xc8_sb[:, :, 1024:2048], xc8_d[:, :, 1024:2048])
            nc.scalar.dma_start(xq8_sb[:, :, 1024:2048], xq8_d[:, :, 1024:2048])
            nc.sync.dma_start(xc8_sb[:, :, 2048:3072], xc8_d[:, :, 2048:3072])
            nc.gpsimd.dma_start(wf_sb[:], wf_d)
            nc.sync.dma_start(xc8_sb[:, :, 3072:4096], xc8_d[:, :, 3072:4096])
            for i in range(QH // 1024):
                s = slice(i * 1024, (i + 1) * 1024)
                nc.gpsimd.dma_start(xqb_sb[:, :, s], xqb_d[:, :, s])
            make_identity(nc, ident[:])
            nc.gpsimd.memset(ut_sb[:, :, C:C + 1], 16.0)

            # score scale: S = (8q . 8k) = 64 s ; softmax wants s/16
            escale = 1.0 / (16.0 * 64.0)

            # one persistent [128, 2048] score-psum (4 banks): S^T chunks
            # write alternating 1024-wide halves, exp reads BOTH halves
            # per call (N=2048 amortizes the ~293ns ACT fixed cost --
            # saves ~9us of scalar-engine time over N=1024 calls)
            sps = mmp.tile([128, 2048], f32, tag="mm", name="sps")

            def emit_st_mms(sb, g):
                """S^T chunk g = K_kt^T Q (fp8 DoubleRow), kt = 2g, 2g+1"""
                qs = slice(sb * 512, (sb + 1) * 512)
                base = (g % 2) * 1024
                for j in range(2):
                    kt = 2 * g + j
                    nc.tensor.matmul(
                        sps[:, base + j * 512:base + (j + 1) * 512],
                        k8_sb[:, :, kt * 128:(kt + 1) * 128],
                        q8_sb[:, :, qs], perf_mode=DR,
                        start=True, stop=True)

            def emit_exp_pair(pt_sb, p):
                """P^T = exp(S^T/1024) -> f8 for chunk pair (2p, 2p+1)"""
                nc.scalar.activation(pt_sb[:, 4 * p:4 * p + 4], sps[:],
                                     AF.Exp, scale=escale)

            def new_pt():
                return ptp.tile([128, NKT, 512], f8, tag="pt", name="pt_sb")

            def qproj_mm(ps, qc):
                s = slice(qc * 512, (qc + 1) * 512)
                for dt in range(CT):
                    nc.tensor.matmul(
                        ps[:, dt * 512:(dt + 1) * 512],
                        wqku_sb[:, 0, :, dt * 128:(dt + 1) * 128],
                        xq8_sb[:, :, s], perf_mode=DR, start=True, stop=True)

            def qproj_copy_dve(ps, qc):
                s = slice(qc * 512, (qc + 1) * 512)
                for dt in range(CT):
                    nc.vector.tensor_scalar_add(
                        q8_sb[:, dt, s], ps[:, dt * 512:(dt + 1) * 512],
                        bias_sb[:, 0, dt:dt + 1])

            # ---- phase 0.  Projections + S^T(0) + U projection.
            # Q/K/U PSUM staging lives in its own scoped pool so the
            # score-psum (mm) rotation is gated only by the exp stream --
            # never by the serial PSUM->SBUF copy queues.  The first q/k
            # copies run on the scalar engine (idle before exp starts);
            # the rest run on the DVE, emitted in consumption order
            # (k-chunk copies round-robined with U-tile casts).
            pt_cur = new_pt()
            with tc.tile_pool(name="up", bufs=4, space="PSUM") as up:
                # Q(qc0) + K(kc0): scalar-engine copies (lead window)
                psq = [up.tile([128, 512], f32, tag="up", name=f"psq{dt}")
                       for dt in range(CT)]
                for dt in range(CT):
                    nc.tensor.matmul(
                        psq[dt][:], wqku_sb[:, 0, :, dt * 128:(dt + 1) * 128],
                        xq8_sb[:, :, 0:512], perf_mode=DR,
                        start=True, stop=True)
                psk = [up.tile([128, 512], f32, tag="up", name=f"psk{dt}")
                       for dt in range(CT)]
                for dt in range(CT):
                    nc.tensor.matmul(
                        psk[dt][:], wqku_sb[:, 1, :, dt * 128:(dt + 1) * 128],
                        xc8_sb[:, :, 0:512], perf_mode=DR,
                        start=True, stop=True)
                for dt in range(CT):
                    nc.scalar.activation(q8_sb[:, dt, 0:512], psq[dt][:],
                                         AF.Identity,
                                         bias=bias_sb[:, 0, dt:dt + 1])
                for dt in range(CT):
                    nc.scalar.activation(k8_sb[:, dt, 0:512], psk[dt][:],
                                         AF.Identity,
                                         bias=bias_sb[:, 1, dt:dt + 1])

                for g in range(NDR):
                    # K projection chunk kc=g+1 (PE), one chunk ahead of
                    # the S^T stream that consumes it
                    if g < KC - 1:
                        kc = g + 1
                        s = slice(kc * 512, (kc + 1) * 512)
                        pk = [up.tile([128, 512], f32, tag="up", name=f"pk{dt}")
                              for dt in range(CT)]
                        for dt in range(CT):
                            nc.tensor.matmul(
                                pk[dt][:],
                                wqku_sb[:, 1, :, dt * 128:(dt + 1) * 128],
                                xc8_sb[:, :, s], perf_mode=DR,
                                start=True, stop=True)
                    emit_st_mms(0, g)
                    if g % 2 == 1:
                        emit_exp_pair(pt_cur, g // 2)
                    # U^T pair (PE) + its fp8 cast (DVE)
                    pu = up.tile([128, 512], f32, tag="up", name="pu")
                    for j in range(2):
                        mt = 2 * g + j
                        nc.tensor.matmul(
                            pu[:, j * 256:(j + 1) * 256],
                            xc8_sb[:, :, mt * 128:(mt + 1) * 128],
                            wqku_sb[:, 2], perf_mode=DR, start=True, stop=True)
                    # DVE queue, consumption order: k-copies then ut-cast
                    if g < KC - 1:
                        kc = g + 1
                        s = slice(kc * 512, (kc + 1) * 512)
                        for dt in range(CT):
                            nc.vector.tensor_scalar_add(
                                k8_sb[:, dt, s], pk[dt][:],
                                bias_sb[:, 1, dt:dt + 1])
                    nc.vector.tensor_copy(
                        ut_sb[:, 2 * g:2 * g + 2, 0:C],
                        pu[:].rearrange("p (j n) -> p j n", j=2))
                # Q(qc1): PE at phase-0 tail, DVE copy after the ut casts
                psq1 = [up.tile([128, 512], f32, tag="up", name=f"psq1{dt}")
                        for dt in range(CT)]
                for dt in range(CT):
                    nc.tensor.matmul(
                        psq1[dt][:], wqku_sb[:, 0, :, dt * 128:(dt + 1) * 128],
                        xq8_sb[:, :, 512:1024], perf_mode=DR,
                        start=True, stop=True)
                for dt in range(CT):
                    nc.vector.tensor_scalar_add(
                        q8_sb[:, dt, 512:1024], psq1[dt][:],
                        bias_sb[:, 0, dt:dt + 1])

            # ---- attention + fused conv, per 512-query superblock,
            # software-pipelined: S^T(sb+1) chunks are interleaved into
            # the front half of each qj-block's PV tile loop; transposes
            # of qj run one block late so the DVE normalize latency never
            # stalls the PE; Wf1 waits until psO's banks are drained.
            with tc.tile_pool(name="pv", bufs=2, space="PSUM") as pvp, \
                 tc.tile_pool(name="po", bufs=1, space="PSUM") as pop:
                for sb in range(NSB):
                    qs = slice(sb * 512, (sb + 1) * 512)
                    pt_sb = pt_cur
                    pt_next = new_pt() if sb + 1 < NSB else None
                    pso = [pop.tile([128, 512], f32, tag=f"po{et}",
                                    name=f"pso{et}") for et in range(CT)]
                    c_blk = [None] * 4

                    # PV: fp8 DoubleRow, P^T tiles stationary (256 keys
                    # each), U^T [keys, 257] moving; [16A | 16R] lands per
                    # 128-query block; normalize on DVE (per-partition
                    # reciprocal+scale)
                    for qj in range(4):
                        psb = pvp.tile([128, C + 1], f32, tag="pv", name="psb")
                        for t in range(NDR):
                            if pt_next is not None and t < 8 and t % 2 == 0:
                                g = 4 * qj + t // 2
                                emit_st_mms(sb + 1, g)
                                if g % 2 == 1:
                                    emit_exp_pair(pt_next, g // 2)
                            nc.tensor.matmul(
                                psb[:],
                                pt_sb[:, 2 * t:2 * t + 2,
                                      qj * 128:(qj + 1) * 128],
                                ut_sb[:, 2 * t:2 * t + 2, 0:C + 1],
                                perf_mode=DR,
                                start=(t == 0), stop=(t == NDR - 1))
                        rinv = cbp.tile([128, 1], f32, tag="rinv", name="rinv")
                        nc.vector.reciprocal(rinv[:], psb[:, C:C + 1])
                        c_sb = cbp.tile([128, C], f16, tag="c", name="c_sb")
                        nc.vector.tensor_scalar_mul(c_sb[:], psb[:, :C],
                                                    rinv[:])
                        c_blk[qj] = c_sb
                        if qj == 0:
                            # conv part 1 (bf16: the direct path needs the
                            # precision).  Emitted after qj0's PV so the
                            # psO banks (freed by sb-1's final adds) are
                            # long since drained; opens the psO group.
                            if sb == 0:
                                # stage Q(qc2/qc3) through the pv pool
                                # (idle buffers here) so the ST(1) chunk
                                # stream's mm rotation is never gated on
                                # these copies
                                for qc in (2, 3):
                                    for dt in range(CT):
                                        s = slice(qc * 512, (qc + 1) * 512)
                                        psx = pvp.tile([128, 512], f32,
                                                       tag="pv", name="psx")
                                        nc.tensor.matmul(
                                            psx[:],
                                            wqku_sb[:, 0, :,
                                                    dt * 128:(dt + 1) * 128],
                                            xq8_sb[:, :, s], perf_mode=DR,
                                            start=True, stop=True)
                                        nc.vector.tensor_scalar_add(
                                            q8_sb[:, dt, s], psx[:],
                                            bias_sb[:, 0, dt:dt + 1])
                            for et in range(CT):
                                for ct in range(CT):
                                    nc.tensor.matmul(
                                        pso[et][:],
                                        wf_sb[:, ct,
                                              et * 128:(et + 1) * 128],
                                        xqb_sb[:, ct, qs],
                                        start=(ct == 0), stop=False)
                        else:
                            # transpose qj-1 into psO via identity-matmul
                            # (one block late: its DVE normalize is done)
                            for et in range(CT):
                                nc.tensor.matmul(
                                    pso[et][:, (qj - 1) * 128:qj * 128],
                                    c_blk[qj - 1][:,
                                                  et * 128:(et + 1) * 128],
                                    ident[:],
                                    start=False, stop=False,
                                    skip_group_check=True)
                    for et in range(CT):
                        nc.tensor.matmul(
                            pso[et][:, 3 * 128:4 * 128],
                            c_blk[3][:, et * 128:(et + 1) * 128],
                            ident[:],
                            start=False, stop=True,
                            skip_group_check=True)

                    # final combine + bias on the vector engine
                    for et in range(CT):
                        o_sb = outp.tile([128, 512], f32, tag="o", name="o_sb")
                        nc.vector.tensor_scalar_add(o_sb[:], pso[et][:],
                                                    bias_sb[:, 2, et:et + 1])
                        nc.sync.dma_start(out_d[:, et, qs], o_sb[:])
                    pt_cur = pt_next
    nc.finalize()
    return nc


def _get_nc():
    if "nc" not in _CACHE:
        _CACHE["nc"] = _build()
    return _CACHE["nc"]


def _in_maps(transformer_features, cnn_features, Wq, bq, Wk, bk, Wv, bv, Wf, bf):
    import ml_dtypes
    f8 = ml_dtypes.float8_e4m3fn

    xt = np.ascontiguousarray(np.asarray(transformer_features, np.float32)
                              .reshape(B, C, N))
    xc = np.ascontiguousarray(np.asarray(cnn_features, np.float32)
                              .reshape(B, C, N))
    Wq = np.asarray(Wq, np.float32)
    Wk = np.asarray(Wk, np.float32)
    Wv = np.asarray(Wv, np.float32)
    Wf = np.asarray(Wf, np.float32)
    bq = np.asarray(bq, np.float32)
    bk = np.asarray(bk, np.float32)
    bv = np.asarray(bv, np.float32)
    bf = np.asarray(bf, np.float32)

    Wf1, Wf2 = Wf[:, :C], Wf[:, C:]
    wq8 = (8.0 * Wq.T).astype(f8)
    wk8 = (8.0 * Wk.T).astype(f8)
    wu8 = (16.0 * (Wf2 @ Wv).T).astype(f8)
    # pack [wq|wk|wu] as [128 part, 3, 2, 256] so they ship as one DMA
    # with 1536B partition rows
    wqku = np.ascontiguousarray(np.stack(
        [w.reshape(CT, 128, C).transpose(1, 0, 2) for w in (wq8, wk8, wu8)],
        axis=1))
    wf1 = np.ascontiguousarray(Wf1.T).astype(ml_dtypes.bfloat16)
    bias3 = np.ascontiguousarray(
        np.stack([8.0 * bq, 8.0 * bk, bf + Wf2 @ bv]))
    xc8 = xc.astype(f8)

    maps = []
    for c in range(NCORES):
        b, h = divmod(c, 2)
        xq = np.ascontiguousarray(xt[b][:, h * QH:(h + 1) * QH])
        maps.append(dict(
            xq8=xq.astype(f8),
            xc8=xc8[b],
            xqb=xq.astype(ml_dtypes.bfloat16),
            wqku=wqku, wf1=wf1,
            bias3=bias3,
        ))
    return maps


def _run(inputs, trace=False):
    from concourse.bass_utils import run_bass_kernel_spmd
    nc = _get_nc()
    maps = _in_maps(**inputs)
    return run_bass_kernel_spmd(nc, maps, list(range(NCORES)), trace=trace)


def kernel(**inputs) -> np.ndarray:
    res = _run(inputs).results
    out = np.empty((B, C, N), np.float32)
    for c in range(NCORES):
        b, h = divmod(c, 2)
        out[b][:, h * QH:(h + 1) * QH] = res[c]["out"]
    return out.reshape(B, C, H, W)


# revision 21
# speedup vs baseline: 1.0435x; 1.0435x over previous
"""CrossAttentionFusion Trainium2 kernel (nn_CrossAttentionFusion__45561013076033).

Full inputs -> full output. Sharding: 8 cores, core c handles batch b=c//2,
query-half h=c%2 (2048 of 4096 queries). Each core holds the full [256,4096]
cnn feature map of its batch (keys), its query-half of the transformer
features, and replicated weights.

Key restructurings vs the naive dataflow:
  * out = Wf1 @ x_trf + Wf2 @ attended + bf'.  Fold Wf2 into the value
    projection: U = (Wf2 @ Wv) @ x_cnn, so attention directly produces
    conv-ready channels; bv's contribution is constant (softmax rows sum
    to 1) and lands in bf' = bf + Wf2 @ bv.
  * Q/K projections, scores, AND the PV matmul all run as fp8e4m3
    DoubleRow matmuls (256-deep contraction in one pass).  exp() writes
    P^T straight to fp8 (logits are bounded ~3, so exp stays far below
    the TRN e4m3 max of 240), and U is quantized to fp8 after its
    on-chip projection; softmax averaging washes the quantization out
    (measured end-to-end rel err 0.0026, same as the f16-P version).
  * Scores are computed pre-transposed, S^T[k, q] = K_kt^T Q, so the P^T
    needed by the PV matmul comes straight out of exp() -- no transpose
    of the [N, N] attention matrix.
  * PV runs with P^T tiles as the *stationary* operand pairs (DoubleRow
    over 256 keys) and U^T [keys, 257] as the moving operand: softmax
    row-sums come free as a 257th column (constant-16: cancels in A/R),
    and normalization stays a cheap per-partition reciprocal+scale.
  * The [q, e] -> [e, q] layout fix-up is a matmul with a 128x128 identity
    as the moving operand, accumulated directly into the Wf1 PSUM group
    (Wf1 runs in bf16 -- the direct conv path needs >=bf16 precision).
  * Q/K projection PSUM->SBUF moves (bias add + fp8 cast) run on the
    vector engine, keeping the scalar engine free for the exp stream
    (exp is the ACT-engine floor at ~73us/core).
"""

import numpy as np

B, C, H, W = 4, 256, 64, 64
N = H * W            # 4096 tokens
NCORES = 8
QH = N // 2          # 2048 queries per core
CT = C // 128        # 2 channel tiles
KC = N // 512        # 8 key chunks of 512
NSB = QH // 512      # 4 superblocks per core
NKT = N // 128       # 32 key tiles
NDR = NKT // 2       # 16 DoubleRow key tiles (256 keys each)
UW = 272             # padded U^T row stride (257 used; 16-aligned)

_CACHE = {}


def _build():
    import concourse.bass as bass
    import concourse.mybir as mybir
    import concourse.tile as tile
    from concourse import bacc
    from concourse.masks import make_identity

    f32 = mybir.dt.float32
    bf16 = mybir.dt.bfloat16
    f16 = mybir.dt.float16
    f8 = mybir.dt.float8e4
    AF = mybir.ActivationFunctionType
    DR = mybir.MatmulPerfMode.DoubleRow

    nc = bacc.Bacc("TRN2", target_bir_lowering=False, debug=False)

    XQ8 = nc.dram_tensor("xq8", [C, QH], f8, kind="ExternalInput")
    XC8 = nc.dram_tensor("xc8", [C, N], f8, kind="ExternalInput")
    XQB = nc.dram_tensor("xqb", [C, QH], bf16, kind="ExternalInput")
    # wq/wk/wu pre-packed on host as [128, 3, 2, 256]: one DMA with
    # 1536B partition rows (separate [256,256] tensors would move as
    # 256B rows -- ~4x the DMA time, each paying the ~2us completion
    # latency)
    WQKU = nc.dram_tensor("wqku", [128, 3, CT, C], f8, kind="ExternalInput")
    WF1 = nc.dram_tensor("wf1", [C, C], bf16, kind="ExternalInput")
    BIAS = nc.dram_tensor("bias3", [3, C], f32, kind="ExternalInput")
    OUT = nc.dram_tensor("out", [C, QH], f32, kind="ExternalOutput")

    xq8_d = XQ8.ap().rearrange("(t p) n -> p t n", p=128)
    xc8_d = XC8.ap().rearrange("(t p) n -> p t n", p=128)
    xqb_d = XQB.ap().rearrange("(t p) n -> p t n", p=128)
    wf_d = WF1.ap().rearrange("(t p) d -> p t d", p=128)
    out_d = OUT.ap().rearrange("(t p) n -> p t n", p=128)

    with tile.TileContext(nc) as tc:
        with tc.tile_pool(name="persist", bufs=1) as per, \
             tc.tile_pool(name="pt", bufs=2) as ptp, \
             tc.tile_pool(name="cb", bufs=4) as cbp, \
             tc.tile_pool(name="outp", bufs=2) as outp, \
             tc.tile_pool(name="mm", bufs=2, space="PSUM") as mmp:

            # ---- persistent tiles ----
            xq8_sb = per.tile([128, CT, QH], f8)
            xc8_sb = per.tile([128, CT, N], f8)
            xqb_sb = per.tile([128, CT, QH], bf16)
            wqku_sb = per.tile([128, 3, CT, C], f8)
            wf_sb = per.tile([128, CT, C], bf16)
            bias_sb = per.tile([128, 3, CT], f32)
            q8_sb = per.tile([128, CT, QH], f8)
            k8_sb = per.tile([128, CT, N], f8)
            ut_sb = per.tile([128, NKT, UW], f8)
            ident = per.tile([128, 128], f16)
            warm_sb = per.tile([1, 1], f32)

            # input DMAs, ordered by first consumer.  1024-column chunks
            # keep per-partition DMA rows >= 1KB (descriptor efficiency);
            # the three bias vectors ride in one packed tensor.
            # preload the exp activation table while the DMAs run (the
            # first real exp would otherwise pay the ~2.7us table switch)
            # spread the input DMAs across the three DMA-capable engine
            # queues (sync/scalar/gpsimd); each dma_start carries a ~2us
            # completion latency, so the lead-critical loads (weights,
            # xq/xc chunk0) get their own queues and everything ships in
            # few, large-row transfers.
            # lead-critical loads ride the two HWDGE queues (sync/scalar)
            # -- the gpsimd queue is SWDGE: its software descriptor
            # generation adds ~5us+ for multi-row patterns, so it only
            # carries the late-needed bulk (wf, xqb).
            nc.scalar.dma_start(wqku_sb[:], WQKU.ap())
            nc.scalar.dma_start(xq8_sb[:, :, 0:1024], xq8_d[:, :, 0:1024])
            nc.sync.dma_start(xc8_sb[:, :, 0:1024], xc8_d[:, :, 0:1024])
            nc.vector.memset(warm_sb[:], 0.0)
            nc.scalar.activation(warm_sb[:], warm_sb[:], AF.Exp, scale=0.0)
            nc.scalar.dma_start(bias_sb[:],
                                BIAS.ap().rearrange("b (t p) -> p b t", p=128))
            nc.sync.dma_start(xc8_sb[:, :, 1024:2048], xc8_d[:, :, 1024:2048])
            nc.scalar.dma_start(xq8_sb[:, :, 1024:2048], xq8_d[:, :, 1024:2048])
            nc.sync.dma_start(xc8_sb[:, :, 2048:3072], xc8_d[:, :, 2048:3072])
            nc.gpsimd.dma_start(wf_sb[:], wf_d)
            nc.sync.dma_start(xc8_sb[:, :, 3072:4096], xc8_d[:, :, 3072:4096])
            for i in range(QH // 1024):
                s = slice(i * 1024, (i + 1) * 1024)
                nc.gpsimd.dma_start(xqb_sb[:, :, s], xqb_d[:, :, s])
            make_identity(nc, ident[:])
            nc.gpsimd.memset(ut_sb[:, :, C:C + 1], 16.0)

            # score scale: S = (8q . 8k) = 64 s ; softmax wants s/16
            escale = 1.0 / (16.0 * 64.0)

            def emit_st_chunk(pt_sb, sb, g0, g1):
                """S^T = K_kt^T Q (fp8 DoubleRow); P^T = exp(S^T/1024) -> f8"""
                qs = slice(sb * 512, (sb + 1) * 512)
                for g in range(g0, g1):
                    ps = mmp.tile([128, 1024], f32, tag="mm", name="ps")
                    for j in range(2):
                        kt = 2 * g + j
                        nc.tensor.matmul(
                            ps[:, j * 512:(j + 1) * 512],
                            k8_sb[:, :, kt * 128:(kt + 1) * 128],
                            q8_sb[:, :, qs], perf_mode=DR,
                            start=True, stop=True)
                    nc.scalar.activation(pt_sb[:, 2 * g:2 * g + 2], ps[:],
                                         AF.Exp, scale=escale)

            def new_pt():
                return ptp.tile([128, NKT, 512], f8, tag="pt", name="pt_sb")

            def qproj_mm(ps, qc):
                s = slice(qc * 512, (qc + 1) * 512)
                for dt in range(CT):
                    nc.tensor.matmul(
                        ps[:, dt * 512:(dt + 1) * 512],
                        wqku_sb[:, 0, :, dt * 128:(dt + 1) * 128],
                        xq8_sb[:, :, s], perf_mode=DR, start=True, stop=True)

            def qproj_copy_dve(ps, qc):
                s = slice(qc * 512, (qc + 1) * 512)
                for dt in range(CT):
                    nc.vector.tensor_scalar_add(
                        q8_sb[:, dt, s], ps[:, dt * 512:(dt + 1) * 512],
                        bias_sb[:, 0, dt:dt + 1])

            # ---- phase 0.  Projections + S^T(0) + U projection.
            # Q/K/U PSUM staging lives in its own scoped pool so the
            # score-psum (mm) rotation is gated only by the exp stream --
            # never by the serial PSUM->SBUF copy queues.  The first q/k
            # copies run on the scalar engine (idle before exp starts);
            # the rest run on the DVE, emitted in consumption order
            # (k-chunk copies round-robined with U-tile casts).
            pt_cur = new_pt()
            with tc.tile_pool(name="up", bufs=4, space="PSUM") as up:
                # Q(qc0) + K(kc0): scalar-engine copies (lead window)
                psq = [up.tile([128, 512], f32, tag="up", name=f"psq{dt}")
                       for dt in range(CT)]
                for dt in range(CT):
                    nc.tensor.matmul(
                        psq[dt][:], wqku_sb[:, 0, :, dt * 128:(dt + 1) * 128],
                        xq8_sb[:, :, 0:512], perf_mode=DR,
                        start=True, stop=True)
                psk = [up.tile([128, 512], f32, tag="up", name=f"psk{dt}")
                       for dt in range(CT)]
                for dt in range(CT):
                    nc.tensor.matmul(
                        psk[dt][:], wqku_sb[:, 1, :, dt * 128:(dt + 1) * 128],
                        xc8_sb[:, :, 0:512], perf_mode=DR,
                        start=True, stop=True)
                for dt in range(CT):
                    nc.scalar.activation(q8_sb[:, dt, 0:512], psq[dt][:],
                                         AF.Identity,
                                         bias=bias_sb[:, 0, dt:dt + 1])
                for dt in range(CT):
                    nc.scalar.activation(k8_sb[:, dt, 0:512], psk[dt][:],
                                         AF.Identity,
                                         bias=bias_sb[:, 1, dt:dt + 1])

                for g in range(NDR):
                    # K projection chunk kc=g+1 (PE), one chunk ahead of
                    # the S^T stream that consumes it
                    if g < KC - 1:
                        kc = g + 1
                        s = slice(kc * 512, (kc + 1) * 512)
                        pk = [up.tile([128, 512], f32, tag="up", name=f"pk{dt}")
                              for dt in range(CT)]
                        for dt in range(CT):
                            nc.tensor.matmul(
                                pk[dt][:],
                                wqku_sb[:, 1, :, dt * 128:(dt + 1) * 128],
                                xc8_sb[:, :, s], perf_mode=DR,
                                start=True, stop=True)
                    emit_st_chunk(pt_cur, 0, g, g + 1)
                    # U^T pair (PE) + its fp8 cast (DVE)
                    pu = up.tile([128, 512], f32, tag="up", name="pu")
                    for j in range(2):
                        mt = 2 * g + j
                        nc.tensor.matmul(
                            pu[:, j * 256:(j + 1) * 256],
                            xc8_sb[:, :, mt * 128:(mt + 1) * 128],
                            wqku_sb[:, 2], perf_mode=DR, start=True, stop=True)
                    # DVE queue, consumption order: k-copies then ut-cast
                    if g < KC - 1:
                        kc = g + 1
                        s = slice(kc * 512, (kc + 1) * 512)
                        for dt in range(CT):
                            nc.vector.tensor_scalar_add(
                                k8_sb[:, dt, s], pk[dt][:],
                                bias_sb[:, 1, dt:dt + 1])
                    nc.vector.tensor_copy(
                        ut_sb[:, 2 * g:2 * g + 2, 0:C],
                        pu[:].rearrange("p (j n) -> p j n", j=2))
                # Q(qc1): PE at phase-0 tail, DVE copy after the ut casts
                psq1 = [up.tile([128, 512], f32, tag="up", name=f"psq1{dt}")
                        for dt in range(CT)]
                for dt in range(CT):
                    nc.tensor.matmul(
                        psq1[dt][:], wqku_sb[:, 0, :, dt * 128:(dt + 1) * 128],
                        xq8_sb[:, :, 512:1024], perf_mode=DR,
                        start=True, stop=True)
                for dt in range(CT):
                    nc.vector.tensor_scalar_add(
                        q8_sb[:, dt, 512:1024], psq1[dt][:],
                        bias_sb[:, 0, dt:dt + 1])

            # ---- attention + fused conv, per 512-query superblock,
            # software-pipelined: S^T(sb+1) chunks are interleaved into
            # the front half of each qj-block's PV tile loop; transposes
            # of qj run one block late so the DVE normalize latency never
            # stalls the PE; Wf1 waits until psO's banks are drained.
            with tc.tile_pool(name="pv", bufs=2, space="PSUM") as pvp, \
                 tc.tile_pool(name="po", bufs=1, space="PSUM") as pop:
                for sb in range(NSB):
                    qs = slice(sb * 512, (sb + 1) * 512)
                    pt_sb = pt_cur
                    pt_next = new_pt() if sb + 1 < NSB else None
                    pso = [pop.tile([128, 512], f32, tag=f"po{et}",
                                    name=f"pso{et}") for et in range(CT)]
                    c_blk = [None] * 4

                    # PV: fp8 DoubleRow, P^T tiles stationary (256 keys
                    # each), U^T [keys, 257] moving; [16A | 16R] lands per
                    # 128-query block; normalize on DVE (per-partition
                    # reciprocal+scale)
                    for qj in range(4):
                        psb = pvp.tile([128, C + 1], f32, tag="pv", name="psb")
                        for t in range(NDR):
                            if pt_next is not None and t < 8 and t % 2 == 0:
                                g = 4 * qj + t // 2
                                emit_st_chunk(pt_next, sb + 1, g, g + 1)
                            nc.tensor.matmul(
                                psb[:],
                                pt_sb[:, 2 * t:2 * t + 2,
                                      qj * 128:(qj + 1) * 128],
                                ut_sb[:, 2 * t:2 * t + 2, 0:C + 1],
                                perf_mode=DR,
                                start=(t == 0), stop=(t == NDR - 1))
                        rinv = cbp.tile([128, 1], f32, tag="rinv", name="rinv")
                        nc.vector.reciprocal(rinv[:], psb[:, C:C + 1])
                        c_sb = cbp.tile([128, C], f16, tag="c", name="c_sb")
                        nc.vector.tensor_scalar_mul(c_sb[:], psb[:, :C],
                                                    rinv[:])
                        c_blk[qj] = c_sb
                        if qj == 0:
                            # conv part 1 (bf16: the direct path needs the
                            # precision).  Emitted after qj0's PV so the
                            # psO banks (freed by sb-1's final adds) are
                            # long since drained; opens the psO group.
                            if sb == 0:
                                # stage Q(qc2/qc3) through the pv pool
                                # (idle buffers here) so the ST(1) chunk
                                # stream's mm rotation is never gated on
                                # these copies
                                for qc in (2, 3):
                                    for dt in range(CT):
                                        s = slice(qc * 512, (qc + 1) * 512)
                                        psx = pvp.tile([128, 512], f32,
                                                       tag="pv", name="psx")
                                        nc.tensor.matmul(
                                            psx[:],
                                            wqku_sb[:, 0, :,
                                                    dt * 128:(dt + 1) * 128],
                                            xq8_sb[:, :, s], perf_mode=DR,
                                            start=True, stop=True)
                                        nc.vector.tensor_scalar_add(
                                            q8_sb[:, dt, s], psx[:],
                                            bias_sb[:, 0, dt:dt + 1])
                            for et in range(CT):
                                for ct in range(CT):
                                    nc.tensor.matmul(
                                        pso[et][:],
                                        wf_sb[:, ct,
                                              et * 128:(et + 1) * 128],
                                        xqb_sb[:, ct, qs],
                                        start=(ct == 0), stop=False)
                        else:
                            # transpose qj-1 into psO via identity-matmul
                            # (one block late: its DVE normalize is done)
                            for et in range(CT):
                                nc.tensor.matmul(
                                    pso[et][:, (qj - 1) * 128:qj * 128],
                                    c_blk[qj - 1][:,
                                                  et * 128:(et + 1) * 128],
                                    ident[:],
                                    start=False, stop=False,
                                    skip_group_check=True)
                    for et in range(CT):
                        nc.tensor.matmul(
                            pso[et][:, 3 * 128:4 * 128],
                            c_blk[3][:, et * 128:(et + 1) * 128],
                            ident[:],
                            start=False, stop=True,
                            skip_group_check=True)

                    # final combine + bias on the vector engine
                    for et in range(CT):
                        o_sb = outp.tile([128, 512], f32, tag="o", name="o_sb")
                        nc.vector.tensor_scalar_add(o_sb[:], pso[et][:],
                                                    bias_sb[:, 2, et:et + 1])
                        nc.sync.dma_start(out_d[:, et, qs], o_sb[:])
                    pt_cur = pt_next
    nc.finalize()
    return nc


def _get_nc():
    if "nc" not in _CACHE:
        _CACHE["nc"] = _build()
    return _CACHE["nc"]


def _in_maps(transformer_features, cnn_features, Wq, bq, Wk, bk, Wv, bv, Wf, bf):
    import ml_dtypes
    f8 = ml_dtypes.float8_e4m3fn

    xt = np.ascontiguousarray(np.asarray(transformer_features, np.float32)
                              .reshape(B, C, N))
    xc = np.ascontiguousarray(np.asarray(cnn_features, np.float32)
                              .reshape(B, C, N))
    Wq = np.asarray(Wq, np.float32)
    Wk = np.asarray(Wk, np.float32)
    Wv = np.asarray(Wv, np.float32)
    Wf = np.asarray(Wf, np.float32)
    bq = np.asarray(bq, np.float32)
    bk = np.asarray(bk, np.float32)
    bv = np.asarray(bv, np.float32)
    bf = np.asarray(bf, np.float32)

    Wf1, Wf2 = Wf[:, :C], Wf[:, C:]
    wq8 = (8.0 * Wq.T).astype(f8)
    wk8 = (8.0 * Wk.T).astype(f8)
    wu8 = (16.0 * (Wf2 @ Wv).T).astype(f8)
    # pack [wq|wk|wu] as [128 part, 3, 2, 256] so they ship as one DMA
    # with 1536B partition rows
    wqku = np.ascontiguousarray(np.stack(
        [w.reshape(CT, 128, C).transpose(1, 0, 2) for w in (wq8, wk8, wu8)],
        axis=1))
    wf1 = np.ascontiguousarray(Wf1.T).astype(ml_dtypes.bfloat16)
    bias3 = np.ascontiguousarray(
        np.stack([8.0 * bq, 8.0 * bk, bf + Wf2 @ bv]))
    xc8 = xc.astype(f8)

    maps = []
    for c in range(NCORES):
        b, h = divmod(c, 2)
        xq = np.ascontiguousarray(xt[b][:, h * QH:(h + 1) * QH])
        maps.append(dict(
            xq8=xq.astype(f8),
            xc8=xc8[b],
            xqb=xq.astype(ml_dtypes.bfloat16),
            wqku=wqku, wf1=wf1,
            bias3=bias3,
        ))
    return maps


def _run(inputs, trace=False):
    from concourse.bass_utils import run_bass_kernel_spmd
    nc = _get_nc()
    maps = _in_maps(**inputs)
    return run_bass_kernel_spmd(nc, maps, list(range(NCORES)), trace=trace)


def kernel(**inputs) -> np.ndarray:
    res = _run(inputs).results
    out = np.empty((B, C, N), np.float32)
    for c in range(NCORES):
        b, h = divmod(c, 2)
        out[b][:, h * QH:(h + 1) * QH] = res[c]["out"]
    return out.reshape(B, C, H, W)


# revision 22
# speedup vs baseline: 1.2585x; 1.2060x over previous
"""CrossAttentionFusion Trainium2 kernel (nn_CrossAttentionFusion__45561013076033).

Full inputs -> full output. Sharding: 8 cores, core c handles batch b=c//2,
query-half h=c%2 (2048 of 4096 queries). Each core holds the full [256,4096]
cnn feature map of its batch (keys), its query-half of the transformer
features, and replicated weights.

Key restructurings vs the naive dataflow:
  * out = Wf1 @ x_trf + Wf2 @ attended + bf'.  Fold Wf2 into the value
    projection: U = (Wf2 @ Wv) @ x_cnn, so attention directly produces
    conv-ready channels; bv's contribution is constant (softmax rows sum
    to 1) and lands in bf' = bf + Wf2 @ bv.
  * Q/K projections, scores, AND the PV matmul all run as fp8e4m3
    DoubleRow matmuls (256-deep contraction in one pass).  exp() writes
    P^T straight to fp8 (logits are bounded ~3, so exp stays far below
    the TRN e4m3 max of 240), and U is quantized to fp8 after its
    on-chip projection; softmax averaging washes the quantization out
    (measured end-to-end rel err 0.0026, same as the f16-P version).
  * Scores are computed pre-transposed, S^T[k, q] = K_kt^T Q, so the P^T
    needed by the PV matmul comes straight out of exp() -- no transpose
    of the [N, N] attention matrix.
  * PV runs with P^T tiles as the *stationary* operand pairs (DoubleRow
    over 256 keys) and U^T [keys, 257] as the moving operand: softmax
    row-sums come free as a 257th column (constant-16: cancels in A/R),
    and normalization stays a cheap per-partition reciprocal+scale.
  * The [q, e] -> [e, q] layout fix-up is a matmul with a 128x128 identity
    as the moving operand, accumulated directly into the Wf1 PSUM group
    (Wf1 runs in bf16 -- the direct conv path needs >=bf16 precision).
  * Q/K projection PSUM->SBUF moves (bias add + fp8 cast) run on the
    vector engine, keeping the scalar engine free for the exp stream
    (exp is the ACT-engine floor at ~73us/core).
"""

import numpy as np

B, C, H, W = 4, 256, 64, 64
N = H * W            # 4096 tokens
NCORES = 8
QH = N // 2          # 2048 queries per core
CT = C // 128        # 2 channel tiles
KC = N // 512        # 8 key chunks of 512
NSB = QH // 512      # 4 superblocks per core
NKT = N // 128       # 32 key tiles
NDR = NKT // 2       # 16 DoubleRow key tiles (256 keys each)
UW = 272             # padded U^T row stride (257 used; 16-aligned)

_CACHE = {}


def _build():
    import concourse.bass as bass
    import concourse.mybir as mybir
    import concourse.tile as tile
    from concourse import bacc
    from concourse.masks import make_identity

    f32 = mybir.dt.float32
    bf16 = mybir.dt.bfloat16
    f16 = mybir.dt.float16
    f8 = mybir.dt.float8e4
    AF = mybir.ActivationFunctionType
    DR = mybir.MatmulPerfMode.DoubleRow

    nc = bacc.Bacc("TRN2", target_bir_lowering=False, debug=False)

    XQ8 = nc.dram_tensor("xq8", [C, QH], f8, kind="ExternalInput")
    XC8 = nc.dram_tensor("xc8", [C, N], f8, kind="ExternalInput")
    XQB = nc.dram_tensor("xqb", [C, QH], bf16, kind="ExternalInput")
    # wq/wk/wu pre-packed on host as [128, 3, 2, 256]: one DMA with
    # 1536B partition rows (separate [256,256] tensors would move as
    # 256B rows -- ~4x the DMA time, each paying the ~2us completion
    # latency)
    WQKU = nc.dram_tensor("wqku", [128, 3, CT, C], f8, kind="ExternalInput")
    WF1 = nc.dram_tensor("wf1", [C, C], bf16, kind="ExternalInput")
    BIAS = nc.dram_tensor("bias3", [3, C], f32, kind="ExternalInput")
    OUT = nc.dram_tensor("out", [C, QH], f32, kind="ExternalOutput")

    xq8_d = XQ8.ap().rearrange("(t p) n -> p t n", p=128)
    xc8_d = XC8.ap().rearrange("(t p) n -> p t n", p=128)
    xqb_d = XQB.ap().rearrange("(t p) n -> p t n", p=128)
    wf_d = WF1.ap().rearrange("(t p) d -> p t d", p=128)
    out_d = OUT.ap().rearrange("(t p) n -> p t n", p=128)

    with tile.TileContext(nc) as tc:
        with tc.tile_pool(name="persist", bufs=1) as per, \
             tc.tile_pool(name="pt", bufs=2) as ptp, \
             tc.tile_pool(name="cb", bufs=4) as cbp, \
             tc.tile_pool(name="outp", bufs=2) as outp, \
             tc.tile_pool(name="mm", bufs=2, space="PSUM") as mmp:

            # ---- persistent tiles ----
            xq8_sb = per.tile([128, CT, QH], f8)
            xc8_sb = per.tile([128, CT, N], f8)
            xqb_sb = per.tile([128, CT, QH], bf16)
            wqku_sb = per.tile([128, 3, CT, C], f8)
            wf_sb = per.tile([128, CT, C], bf16)
            bias_sb = per.tile([128, 3, CT], f32)
            q8_sb = per.tile([128, CT, QH], f8)
            k8_sb = per.tile([128, CT, N], f8)
            ut_sb = per.tile([128, NKT, UW], f8)
            ident = per.tile([128, 128], f16)
            warm_sb = per.tile([1, 1], f32)

            # input DMAs, ordered by first consumer.  1024-column chunks
            # keep per-partition DMA rows >= 1KB (descriptor efficiency);
            # the three bias vectors ride in one packed tensor.
            # preload the exp activation table while the DMAs run (the
            # first real exp would otherwise pay the ~2.7us table switch)
            # spread the input DMAs across the three DMA-capable engine
            # queues (sync/scalar/gpsimd); each dma_start carries a ~2us
            # completion latency, so the lead-critical loads (weights,
            # xq/xc chunk0) get their own queues and everything ships in
            # few, large-row transfers.
            # All SDMA queues share the HBM bandwidth (packet-granular
            # round-robin), so parallel queues make critical transfers
            # finish LATE.  Instead: the two small critical loads (packed
            # weights + bias) ride the scalar HWDGE queue; everything
            # else is SERIAL on the sync queue in consumption order --
            # per-queue FIFO is the priority mechanism.
            nc.scalar.dma_start(wqku_sb[:], WQKU.ap())
            nc.scalar.dma_start(bias_sb[:],
                                BIAS.ap().rearrange("b (t p) -> p b t", p=128))
            nc.sync.dma_start(xq8_sb[:, :, 0:1024], xq8_d[:, :, 0:1024])
            nc.sync.dma_start(xc8_sb[:, :, 0:1024], xc8_d[:, :, 0:1024])
            nc.vector.memset(warm_sb[:], 0.0)
            nc.scalar.activation(warm_sb[:], warm_sb[:], AF.Exp, scale=0.0)
            nc.sync.dma_start(xc8_sb[:, :, 1024:2048], xc8_d[:, :, 1024:2048])
            nc.sync.dma_start(xc8_sb[:, :, 2048:3072], xc8_d[:, :, 2048:3072])
            nc.sync.dma_start(xc8_sb[:, :, 3072:4096], xc8_d[:, :, 3072:4096])
            nc.sync.dma_start(xq8_sb[:, :, 1024:2048], xq8_d[:, :, 1024:2048])
            for i in range(QH // 1024):
                s = slice(i * 1024, (i + 1) * 1024)
                nc.sync.dma_start(xqb_sb[:, :, s], xqb_d[:, :, s])
            nc.sync.dma_start(wf_sb[:], wf_d)
            make_identity(nc, ident[:])
            nc.gpsimd.memset(ut_sb[:, :, C:C + 1], 16.0)

            # score scale: S = (8q . 8k) = 64 s ; softmax wants s/16
            escale = 1.0 / (16.0 * 64.0)

            def emit_st_chunk(pt_sb, sb, g0, g1):
                """S^T = K_kt^T Q (fp8 DoubleRow); P^T = exp(S^T/1024) -> f8"""
                qs = slice(sb * 512, (sb + 1) * 512)
                for g in range(g0, g1):
                    ps = mmp.tile([128, 1024], f32, tag="mm", name="ps")
                    for j in range(2):
                        kt = 2 * g + j
                        nc.tensor.matmul(
                            ps[:, j * 512:(j + 1) * 512],
                            k8_sb[:, :, kt * 128:(kt + 1) * 128],
                            q8_sb[:, :, qs], perf_mode=DR,
                            start=True, stop=True)
                    nc.scalar.activation(pt_sb[:, 2 * g:2 * g + 2], ps[:],
                                         AF.Exp, scale=escale)

            def new_pt():
                return ptp.tile([128, NKT, 512], f8, tag="pt", name="pt_sb")

            def qproj_mm(ps, qc):
                s = slice(qc * 512, (qc + 1) * 512)
                for dt in range(CT):
                    nc.tensor.matmul(
                        ps[:, dt * 512:(dt + 1) * 512],
                        wqku_sb[:, 0, :, dt * 128:(dt + 1) * 128],
                        xq8_sb[:, :, s], perf_mode=DR, start=True, stop=True)

            def qproj_copy_dve(ps, qc):
                s = slice(qc * 512, (qc + 1) * 512)
                for dt in range(CT):
                    nc.vector.tensor_scalar_add(
                        q8_sb[:, dt, s], ps[:, dt * 512:(dt + 1) * 512],
                        bias_sb[:, 0, dt:dt + 1])

            # ---- phase 0.  Projections + S^T(0) + U projection.
            # Q/K/U PSUM staging lives in its own scoped pool so the
            # score-psum (mm) rotation is gated only by the exp stream --
            # never by the serial PSUM->SBUF copy queues.  The first q/k
            # copies run on the scalar engine (idle before exp starts);
            # the rest run on the DVE, emitted in consumption order
            # (k-chunk copies round-robined with U-tile casts).
            pt_cur = new_pt()
            with tc.tile_pool(name="up", bufs=4, space="PSUM") as up:
                # Q(qc0) + K(kc0): scalar-engine copies (lead window)
                psq = [up.tile([128, 512], f32, tag="up", name=f"psq{dt}")
                       for dt in range(CT)]
                for dt in range(CT):
                    nc.tensor.matmul(
                        psq[dt][:], wqku_sb[:, 0, :, dt * 128:(dt + 1) * 128],
                        xq8_sb[:, :, 0:512], perf_mode=DR,
                        start=True, stop=True)
                psk = [up.tile([128, 512], f32, tag="up", name=f"psk{dt}")
                       for dt in range(CT)]
                for dt in range(CT):
                    nc.tensor.matmul(
                        psk[dt][:], wqku_sb[:, 1, :, dt * 128:(dt + 1) * 128],
                        xc8_sb[:, :, 0:512], perf_mode=DR,
                        start=True, stop=True)
                for dt in range(CT):
                    nc.scalar.activation(q8_sb[:, dt, 0:512], psq[dt][:],
                                         AF.Identity,
                                         bias=bias_sb[:, 0, dt:dt + 1])
                for dt in range(CT):
                    nc.scalar.activation(k8_sb[:, dt, 0:512], psk[dt][:],
                                         AF.Identity,
                                         bias=bias_sb[:, 1, dt:dt + 1])

                for g in range(NDR):
                    # K projection chunk kc=g+1 (PE), one chunk ahead of
                    # the S^T stream that consumes it
                    if g < KC - 1:
                        kc = g + 1
                        s = slice(kc * 512, (kc + 1) * 512)
                        pk = [up.tile([128, 512], f32, tag="up", name=f"pk{dt}")
                              for dt in range(CT)]
                        for dt in range(CT):
                            nc.tensor.matmul(
                                pk[dt][:],
                                wqku_sb[:, 1, :, dt * 128:(dt + 1) * 128],
                                xc8_sb[:, :, s], perf_mode=DR,
                                start=True, stop=True)
                    emit_st_chunk(pt_cur, 0, g, g + 1)
                    # U^T pair (PE) + its fp8 cast (DVE)
                    pu = up.tile([128, 512], f32, tag="up", name="pu")
                    for j in range(2):
                        mt = 2 * g + j
                        nc.tensor.matmul(
                            pu[:, j * 256:(j + 1) * 256],
                            xc8_sb[:, :, mt * 128:(mt + 1) * 128],
                            wqku_sb[:, 2], perf_mode=DR, start=True, stop=True)
                    # DVE queue, consumption order: k-copies then ut-cast
                    if g < KC - 1:
                        kc = g + 1
                        s = slice(kc * 512, (kc + 1) * 512)
                        for dt in range(CT):
                            nc.vector.tensor_scalar_add(
                                k8_sb[:, dt, s], pk[dt][:],
                                bias_sb[:, 1, dt:dt + 1])
                    nc.vector.tensor_copy(
                        ut_sb[:, 2 * g:2 * g + 2, 0:C],
                        pu[:].rearrange("p (j n) -> p j n", j=2))
                # Q(qc1): PE at phase-0 tail, DVE copy after the ut casts
                psq1 = [up.tile([128, 512], f32, tag="up", name=f"psq1{dt}")
                        for dt in range(CT)]
                for dt in range(CT):
                    nc.tensor.matmul(
                        psq1[dt][:], wqku_sb[:, 0, :, dt * 128:(dt + 1) * 128],
                        xq8_sb[:, :, 512:1024], perf_mode=DR,
                        start=True, stop=True)
                for dt in range(CT):
                    nc.vector.tensor_scalar_add(
                        q8_sb[:, dt, 512:1024], psq1[dt][:],
                        bias_sb[:, 0, dt:dt + 1])

            # ---- attention + fused conv, per 512-query superblock,
            # software-pipelined: S^T(sb+1) chunks are interleaved into
            # the front half of each qj-block's PV tile loop; transposes
            # of qj run one block late so the DVE normalize latency never
            # stalls the PE; Wf1 waits until psO's banks are drained.
            with tc.tile_pool(name="pv", bufs=2, space="PSUM") as pvp, \
                 tc.tile_pool(name="po", bufs=1, space="PSUM") as pop:
                for sb in range(NSB):
                    qs = slice(sb * 512, (sb + 1) * 512)
                    pt_sb = pt_cur
                    pt_next = new_pt() if sb + 1 < NSB else None
                    pso = [pop.tile([128, 512], f32, tag=f"po{et}",
                                    name=f"pso{et}") for et in range(CT)]
                    c_blk = [None] * 4

                    # PV: fp8 DoubleRow, P^T tiles stationary (256 keys
                    # each), U^T [keys, 257] moving; [16A | 16R] lands per
                    # 128-query block; normalize on DVE (per-partition
                    # reciprocal+scale)
                    for qj in range(4):
                        psb = pvp.tile([128, C + 1], f32, tag="pv", name="psb")
                        for t in range(NDR):
                            if pt_next is not None and t < 8 and t % 2 == 0:
                                g = 4 * qj + t // 2
                                emit_st_chunk(pt_next, sb + 1, g, g + 1)
                            nc.tensor.matmul(
                                psb[:],
                                pt_sb[:, 2 * t:2 * t + 2,
                                      qj * 128:(qj + 1) * 128],
                                ut_sb[:, 2 * t:2 * t + 2, 0:C + 1],
                                perf_mode=DR,
                                start=(t == 0), stop=(t == NDR - 1))
                        rinv = cbp.tile([128, 1], f32, tag="rinv", name="rinv")
                        nc.vector.reciprocal(rinv[:], psb[:, C:C + 1])
                        c_sb = cbp.tile([128, C], f16, tag="c", name="c_sb")
                        nc.vector.tensor_scalar_mul(c_sb[:], psb[:, :C],
                                                    rinv[:])
                        c_blk[qj] = c_sb
                        if qj == 0:
                            # conv part 1 (bf16: the direct path needs the
                            # precision).  Emitted after qj0's PV so the
                            # psO banks (freed by sb-1's final adds) are
                            # long since drained; opens the psO group.
                            if sb == 0:
                                # stage Q(qc2/qc3) through the pv pool
                                # (idle buffers here) so the ST(1) chunk
                                # stream's mm rotation is never gated on
                                # these copies
                                for qc in (2, 3):
                                    for dt in range(CT):
                                        s = slice(qc * 512, (qc + 1) * 512)
                                        psx = pvp.tile([128, 512], f32,
                                                       tag="pv", name="psx")
                                        nc.tensor.matmul(
                                            psx[:],
                                            wqku_sb[:, 0, :,
                                                    dt * 128:(dt + 1) * 128],
                                            xq8_sb[:, :, s], perf_mode=DR,
                                            start=True, stop=True)
                                        nc.vector.tensor_scalar_add(
                                            q8_sb[:, dt, s], psx[:],
                                            bias_sb[:, 0, dt:dt + 1])
                            for et in range(CT):
                                for ct in range(CT):
                                    nc.tensor.matmul(
                                        pso[et][:],
                                        wf_sb[:, ct,
                                              et * 128:(et + 1) * 128],
                                        xqb_sb[:, ct, qs],
                                        start=(ct == 0), stop=False)
                        else:
                            # transpose qj-1 into psO via identity-matmul
                            # (one block late: its DVE normalize is done)
                            for et in range(CT):
                                nc.tensor.matmul(
                                    pso[et][:, (qj - 1) * 128:qj * 128],
                                    c_blk[qj - 1][:,
                                                  et * 128:(et + 1) * 128],
                                    ident[:],
                                    start=False, stop=False,
                                    skip_group_check=True)
                    for et in range(CT):
                        nc.tensor.matmul(
                            pso[et][:, 3 * 128:4 * 128],
                            c_blk[3][:, et * 128:(et + 1) * 128],
                            ident[:],
                            start=False, stop=True,
                            skip_group_check=True)

                    # final combine + bias on the vector engine
                    for et in range(CT):
                        o_sb = outp.tile([128, 512], f32, tag="o", name="o_sb")
                        nc.vector.tensor_scalar_add(o_sb[:], pso[et][:],
                                                    bias_sb[:, 2, et:et + 1])
                        nc.sync.dma_start(out_d[:, et, qs], o_sb[:])
                    pt_cur = pt_next
    nc.finalize()
    return nc


def _get_nc():
    if "nc" not in _CACHE:
        _CACHE["nc"] = _build()
    return _CACHE["nc"]


def _in_maps(transformer_features, cnn_features, Wq, bq, Wk, bk, Wv, bv, Wf, bf):
    import ml_dtypes
    f8 = ml_dtypes.float8_e4m3fn

    xt = np.ascontiguousarray(np.asarray(transformer_features, np.float32)
                              .reshape(B, C, N))
    xc = np.ascontiguousarray(np.asarray(cnn_features, np.float32)
                              .reshape(B, C, N))
    Wq = np.asarray(Wq, np.float32)
    Wk = np.asarray(Wk, np.float32)
    Wv = np.asarray(Wv, np.float32)
    Wf = np.asarray(Wf, np.float32)
    bq = np.asarray(bq, np.float32)
    bk = np.asarray(bk, np.float32)
    bv = np.asarray(bv, np.float32)
    bf = np.asarray(bf, np.float32)

    Wf1, Wf2 = Wf[:, :C], Wf[:, C:]
    wq8 = (8.0 * Wq.T).astype(f8)
    wk8 = (8.0 * Wk.T).astype(f8)
    wu8 = (16.0 * (Wf2 @ Wv).T).astype(f8)
    # pack [wq|wk|wu] as [128 part, 3, 2, 256] so they ship as one DMA
    # with 1536B partition rows
    wqku = np.ascontiguousarray(np.stack(
        [w.reshape(CT, 128, C).transpose(1, 0, 2) for w in (wq8, wk8, wu8)],
        axis=1))
    wf1 = np.ascontiguousarray(Wf1.T).astype(ml_dtypes.bfloat16)
    bias3 = np.ascontiguousarray(
        np.stack([8.0 * bq, 8.0 * bk, bf + Wf2 @ bv]))
    xc8 = xc.astype(f8)

    maps = []
    for c in range(NCORES):
        b, h = divmod(c, 2)
        xq = np.ascontiguousarray(xt[b][:, h * QH:(h + 1) * QH])
        maps.append(dict(
            xq8=xq.astype(f8),
            xc8=xc8[b],
            xqb=xq.astype(ml_dtypes.bfloat16),
            wqku=wqku, wf1=wf1,
            bias3=bias3,
        ))
    return maps


def _run(inputs, trace=False):
    from concourse.bass_utils import run_bass_kernel_spmd
    nc = _get_nc()
    maps = _in_maps(**inputs)
    return run_bass_kernel_spmd(nc, maps, list(range(NCORES)), trace=trace)


def kernel(**inputs) -> np.ndarray:
    res = _run(inputs).results
    out = np.empty((B, C, N), np.float32)
    for c in range(NCORES):
        b, h = divmod(c, 2)
        out[b][:, h * QH:(h + 1) * QH] = res[c]["out"]
    return out.reshape(B, C, H, W)
